# revision 3
# baseline (speedup 1.0000x reference)
"""Trainium2 Bass kernel for the CliffordKAN layer problem (fp8 rework).

Restructure vs kernel.py: the big contraction's stationary operand is a
(128 g, 128 ox-half) slab of W in fp8-e3m4 (1 byte -> halves HBM
traffic), and the moving operand is the fp16 rbf chunk (128 g, 64 b).
PSUM output is (128 ox-half, 64 b) -> the PE runs with all 128 output
partitions occupied (baseline used only 64 = B), i.e. 2x MACs/cycle.
Per k-tile: 2 matmuls (ox halves) x 64 moving rows = 128 cycles vs the
baseline's 256.

Host reassembles out[oxl, h, b] -> out[b, 128h+oxl].

Vs the fp16 baseline: W is stored/streamed as fp8-e3m4 (4 mantissa
bits; rel err ~1.1e-2, gate 2e-2), halving HBM traffic 16.8->8.4 MB
per core, and the big matmuls run with 128 PSUM output partitions
(stationary = W half-tile, moving = rbf chunk) so the moving-stream
total halves: 32768 rows vs 65536.  Measured steady state (For_i
delta, 8 tunneled cores): 41.7 us/iter vs 46.4 baseline; single-shot
stream-bound estimate ~18-20 us.
"""

import numpy as np

from concourse import bacc, bass, mybir  # noqa: F401
from concourse.bass_utils import run_bass_kernel_spmd
from concourse.tile import TileContext

B, I, O, G, X = 64, 64, 64, 4096, 4
NCORES = 8
GS = G // NCORES            # grid points per core = 512
NGB = GS // 128             # g-blocks per core = 4
NKT = NGB * I               # big-matmul k-tiles per core = 256
TPB = 32                    # k-tiles per DMA batch (= 1 MB in fp8)
NBLK = NKT // TPB           # 8 weight DMA batches
OX = O * X                  # 256
IB = I * B                  # 4096
NCH = IB // 512             # rbf chunks (N=512 matmuls) per g-block = 8
IPC = 512 // B              # i's per rbf chunk = 8

_nc_cache = None
last_results = None


def _cayley():
    C = np.zeros((4, 4, 4), dtype=np.float32)
    entries = [
        (0, 0, 0, 1), (0, 1, 1, 1), (0, 2, 2, 1), (0, 3, 3, 1),
        (1, 0, 1, 1), (1, 1, 0, 1), (1, 2, 3, 1), (1, 3, 2, 1),
        (2, 0, 2, 1), (2, 1, 3, -1), (2, 2, 0, 1), (2, 3, 1, -1),
        (3, 0, 3, 1), (3, 1, 2, -1), (3, 2, 1, 1), (3, 3, 0, -1),
    ]
    for xx, y, z, s in entries:
        C[xx, y, z] = s
    return C


def _build_bass(reps=1, loop_n=0):
    global _nc_cache
    if reps == 1 and loop_n == 0 and _nc_cache is not None:
        return _nc_cache

    nc = bacc.Bacc(
        "TRN2", target_bir_lowering=False, debug=False, num_devices=NCORES
    )
    f32 = mybir.dt.float32
    bf16 = mybir.dt.bfloat16
    f16 = mybir.dt.float16
    f8 = mybir.dt.float8e3

    wt = nc.dram_tensor("wt", [NBLK, 128, TPB, OX], f8, kind="ExternalInput")
    ga = nc.dram_tensor("ga", [24, GS], bf16, kind="ExternalInput")
    xa = nc.dram_tensor("xa", [24, IB], bf16, kind="ExternalInput")
    ls = nc.dram_tensor("ls", [128, 3, B], f16, kind="ExternalInput")
    ws = nc.dram_tensor("ws", [128, 3, OX], f16, kind="ExternalInput")
    out = nc.dram_tensor("out", [128, 2, B], f32, kind="ExternalOutput")

    with TileContext(nc) as tc:
        with (
            tc.tile_pool(name="const", bufs=1) as const,
            tc.tile_pool(name="wpool", bufs=8) as wpool,
            tc.tile_pool(name="rpool", bufs=3) as rpool,
            tc.tile_pool(name="psa", bufs=2, space="PSUM") as psa_pool,
            tc.tile_pool(name="pso", bufs=1, space="PSUM") as pso_pool,
        ):
            ga_t = const.tile([24, GS], bf16)
            nc.sync.dma_start(ga_t[:], ga[:])
            xa_t = const.tile([24, IB], bf16)
            nc.sync.dma_start(xa_t[:], xa[:])
            ls_t = const.tile([128, 3, B], f16)
            nc.sync.dma_start(ls_t[:], ls[:])
            ws_t = const.tile([128, 3, OX], f16)
            nc.sync.dma_start(ws_t[:], ws[:])

            pso = pso_pool.tile([128, 2, B], f32)

            def small_chunk(gb, nb):
                psa = psa_pool.tile([128, 512], f32)
                nc.tensor.matmul(
                    psa[:],
                    ga_t[:, gb * 128:(gb + 1) * 128],
                    xa_t[:, nb * 512:(nb + 1) * 512],
                    start=True,
                    stop=True,
                )
                rbf = rpool.tile([128, 512], f16)
                nc.scalar.activation(
                    rbf[:], psa[:], mybir.ActivationFunctionType.Exp
                )
                return rbf

            chunks = [(gb, nb) for gb in range(NGB) for nb in range(NCH)]

            def body():
                rbf_next = small_chunk(*chunks[0])
                q = 0
                w_t = None
                for idx in range(len(chunks)):
                    rbf = rbf_next
                    if idx + 1 < len(chunks):
                        rbf_next = small_chunk(*chunks[idx + 1])
                    for il in range(IPC):
                        blk, t = divmod(q, TPB)
                        if t == 0:
                            w_t = wpool.tile([128, TPB, OX], f8)
                            nc.sync.dma_start(w_t[:], wt[blk])
                        for h in range(2):
                            # one start only: the pending-zero marking is
                            # per 2KB zero region, so a second start=True
                            # (h=1) would re-mark h=0's bytes and turn the
                            # next h=0 accumulate into an overwrite.
                            nc.tensor.matmul(
                                pso[:, h, :],
                                w_t[:, t, h * 128:(h + 1) * 128],
                                rbf[:, il * B:(il + 1) * B],
                                start=(q == 0 and h == 0),
                                stop=False,
                                skip_group_check=True,
                            )
                        q += 1
                for s in range(3):
                    for h in range(2):
                        nc.tensor.matmul(
                            pso[:, h, :],
                            ws_t[:, s, h * 128:(h + 1) * 128],
                            ls_t[:, s, :],
                            start=False,
                            stop=(s == 2),
                            skip_group_check=True,
                        )

            if loop_n > 0:
                with tc.For_i(0, loop_n, 1):
                    body()
            else:
                for _rep in range(reps):
                    body()
            out_t = const.tile([128, 2, B], f32)
            nc.vector.tensor_copy(out_t[:], pso[:])
            nc.sync.dma_start(out[:], out_t[:])

    nc.compile()
    _nc_cache = nc
    return nc


def make_core_inputs(x, grid, weights, silu_weight, silu_bias):
    """Host-side shard + layout prep. Returns list of 8 input dicts."""
    x = np.ascontiguousarray(x, dtype=np.float32)
    grid = np.ascontiguousarray(grid, dtype=np.float32)
    weights = np.ascontiguousarray(weights, dtype=np.float32)
    silu_weight = np.ascontiguousarray(silu_weight, dtype=np.float32)
    silu_bias = np.ascontiguousarray(silu_bias, dtype=np.float32)

    import ml_dtypes

    def split24(a6, pattern):
        hi = a6.astype(ml_dtypes.bfloat16)
        lo = (a6 - hi.astype(np.float32)).astype(ml_dtypes.bfloat16)
        parts = {"h": hi, "l": lo}
        return np.ascontiguousarray(
            np.concatenate([parts[p] for p in pattern], axis=0)
        )

    # xa: (6, I*B), column j = i*B + b
    xt = x.transpose(1, 0, 2)                       # (I, B, X)
    xa = np.empty((6, IB), dtype=np.float32)
    xa[0:4] = xt.reshape(IB, X).T
    xa[4] = 1.0
    xa[5] = -(xt ** 2).sum(-1).reshape(IB)
    xa24 = split24(xa, "hhll")

    # silu lhsT (core 0 only): rows k2 = i*4+y -> silu(x)[b,i,y]; row 256 -> 1
    sx = x / (1.0 + np.exp(-x))                     # silu(x), (B, I, X)
    lsf = np.zeros((384, B), dtype=np.float32)
    lsf[0:256] = sx.transpose(1, 2, 0).reshape(256, B)
    lsf[256] = 1.0
    ls0 = np.ascontiguousarray(
        lsf.reshape(3, 128, B).transpose(1, 0, 2)).astype(np.float16)
    lsz = np.zeros_like(ls0)

    # silu rhs: M2[(i,y),(o,z)] = sum_x silu_weight[i,o,x]*C[x,y,z]; row 256 bias
    C = _cayley()
    m2 = np.einsum("iox,xyz->iyoz", silu_weight, C).reshape(256, OX)
    wsf = np.zeros((384, OX), dtype=np.float32)
    wsf[0:256] = m2
    wsf[256] = silu_bias.sum(axis=0).reshape(OX)
    ws = np.ascontiguousarray(
        wsf.reshape(3, 128, OX).transpose(1, 0, 2)).astype(np.float16)

    in_maps = []
    for c in range(NCORES):
        gsl = slice(c * GS, (c + 1) * GS)
        gc = grid[gsl]                              # (GS, 4)
        ga = np.empty((6, GS), dtype=np.float32)
        ga[0:4] = 2.0 * gc.T
        ga[4] = -(gc ** 2).sum(-1)
        ga[5] = 1.0
        ga24 = split24(ga, "hlhl")

        # W slab -> [blk, p, t, c] with k-tile q = gb*I + i, rows p = g in block
        warr = weights[:, :, gsl, :].transpose(0, 2, 1, 3).reshape(I, GS, OX)
        tmp = warr.reshape(I, NGB, 128, OX).transpose(1, 0, 2, 3)
        tmp = tmp.reshape(NBLK, TPB, 128, OX).transpose(0, 2, 1, 3)
        wtc = np.ascontiguousarray(tmp).astype(ml_dtypes.float8_e3m4)

        in_maps.append({
            "wt": wtc,
            "ga": ga24,
            "xa": xa24,
            "ls": ls0 if c == 0 else lsz,
            "ws": ws,
        })
    return in_maps


def kernel(x, grid, weights, silu_weight, silu_bias):
    global last_results
    nc = _build_bass()
    in_maps = make_core_inputs(x, grid, weights, silu_weight, silu_bias)
    res = run_bass_kernel_spmd(nc, in_maps, list(range(NCORES)))
    last_results = res
    acc = np.zeros((B, OX), dtype=np.float32)
    for r in res.results:
        o = r["out"]                                # (128, 2, B)
        acc += o.transpose(2, 1, 0).reshape(B, OX)
    return acc.reshape(B, O, X)


# revision 4
# speedup vs baseline: 1.0319x; 1.0319x over previous
"""Trainium2 Bass kernel for the CliffordKAN layer problem (fp8 rework).

Restructure vs kernel.py: the big contraction's stationary operand is a
(128 g, 128 ox-half) slab of W in fp8-e3m4 (1 byte -> halves HBM
traffic), and the moving operand is the fp16 rbf chunk (128 g, 64 b).
PSUM output is (128 ox-half, 64 b) -> the PE runs with all 128 output
partitions occupied (baseline used only 64 = B), i.e. 2x MACs/cycle.
Per k-tile: 2 matmuls (ox halves) x 64 moving rows = 128 cycles vs the
baseline's 256.

Host reassembles out[oxl, h, b] -> out[b, 128h+oxl].

Vs the fp16 baseline: W is stored/streamed as fp8-e3m4 (4 mantissa
bits; rel err ~1.1e-2, gate 2e-2), halving HBM traffic 16.8->8.4 MB
per core, and the big matmuls run with 128 PSUM output partitions
(stationary = W half-tile, moving = rbf chunk) so the moving-stream
total halves: 32768 rows vs 65536.  Measured steady state (For_i
delta, 8 tunneled cores): 41.7 us/iter vs 46.4 baseline; single-shot
stream-bound estimate ~18-20 us.
"""

import numpy as np

from concourse import bacc, bass, mybir  # noqa: F401
from concourse.bass_utils import run_bass_kernel_spmd
from concourse.tile import TileContext

B, I, O, G, X = 64, 64, 64, 4096, 4
NCORES = 8
GS = G // NCORES            # grid points per core = 512
NGB = GS // 128             # g-blocks per core = 4
NKT = NGB * I               # big-matmul k-tiles per core = 256
TPB = 32                    # k-tiles per DMA batch (= 1 MB in fp8)
NBLK = NKT // TPB           # 8 weight DMA batches
OX = O * X                  # 256
IB = I * B                  # 4096
NCH = IB // 512             # rbf chunks (N=512 matmuls) per g-block = 8
IPC = 512 // B              # i's per rbf chunk = 8

_nc_cache = None
last_results = None


def _cayley():
    C = np.zeros((4, 4, 4), dtype=np.float32)
    entries = [
        (0, 0, 0, 1), (0, 1, 1, 1), (0, 2, 2, 1), (0, 3, 3, 1),
        (1, 0, 1, 1), (1, 1, 0, 1), (1, 2, 3, 1), (1, 3, 2, 1),
        (2, 0, 2, 1), (2, 1, 3, -1), (2, 2, 0, 1), (2, 3, 1, -1),
        (3, 0, 3, 1), (3, 1, 2, -1), (3, 2, 1, 1), (3, 3, 0, -1),
    ]
    for xx, y, z, s in entries:
        C[xx, y, z] = s
    return C


def _build_bass(reps=1, loop_n=0):
    global _nc_cache
    if reps == 1 and loop_n == 0 and _nc_cache is not None:
        return _nc_cache

    nc = bacc.Bacc(
        "TRN2", target_bir_lowering=False, debug=False, num_devices=NCORES
    )
    f32 = mybir.dt.float32
    bf16 = mybir.dt.bfloat16
    f16 = mybir.dt.float16
    f8 = mybir.dt.float8e3

    wt = nc.dram_tensor("wt", [NBLK, 128, TPB, OX], f8, kind="ExternalInput")
    ga = nc.dram_tensor("ga", [24, GS], bf16, kind="ExternalInput")
    xa = nc.dram_tensor("xa", [24, IB], bf16, kind="ExternalInput")
    ls = nc.dram_tensor("ls", [128, 3, B], f16, kind="ExternalInput")
    ws = nc.dram_tensor("ws", [128, 3, OX], f16, kind="ExternalInput")
    out = nc.dram_tensor("out", [128, 2, B], f32, kind="ExternalOutput")

    with TileContext(nc) as tc:
        with (
            tc.tile_pool(name="const", bufs=1) as const,
            tc.tile_pool(name="wpool", bufs=8) as wpool,
            tc.tile_pool(name="rpool", bufs=3) as rpool,
            tc.tile_pool(name="psa", bufs=3, space="PSUM") as psa_pool,
            tc.tile_pool(name="pso", bufs=1, space="PSUM") as pso_pool,
        ):
            ga_t = const.tile([24, GS], bf16)
            nc.sync.dma_start(ga_t[:], ga[:])
            xa_t = const.tile([24, IB], bf16)
            nc.sync.dma_start(xa_t[:], xa[:])
            ls_t = const.tile([128, 3, B], f16)
            nc.sync.dma_start(ls_t[:], ls[:])
            ws_t = const.tile([128, 3, OX], f16)
            nc.sync.dma_start(ws_t[:], ws[:])

            pso = pso_pool.tile([128, 2, B], f32)

            def small_chunk(gb, nb):
                psa = psa_pool.tile([128, 512], f32)
                nc.tensor.matmul(
                    psa[:],
                    ga_t[:, gb * 128:(gb + 1) * 128],
                    xa_t[:, nb * 512:(nb + 1) * 512],
                    start=True,
                    stop=True,
                )
                rbf = rpool.tile([128, 512], f16)
                nc.scalar.activation(
                    rbf[:], psa[:], mybir.ActivationFunctionType.Exp
                )
                return rbf

            chunks = [(gb, nb) for gb in range(NGB) for nb in range(NCH)]

            def body():
                # 2-chunk rbf lookahead: ScalarE's exp (~600ns) is as long
                # as a chunk's PE work, so 1-deep pipelining leaves exp on
                # the critical path between chunks.
                rbf_q = [small_chunk(*chunks[0]), small_chunk(*chunks[1])]
                q = 0
                w_t = None
                for idx in range(len(chunks)):
                    rbf = rbf_q.pop(0)
                    if idx + 2 < len(chunks):
                        rbf_q.append(small_chunk(*chunks[idx + 2]))
                    for il in range(IPC):
                        blk, t = divmod(q, TPB)
                        if t == 0:
                            w_t = wpool.tile([128, TPB, OX], f8)
                            nc.sync.dma_start(w_t[:], wt[blk])
                        for h in range(2):
                            # one start only: the pending-zero marking is
                            # per 2KB zero region, so a second start=True
                            # (h=1) would re-mark h=0's bytes and turn the
                            # next h=0 accumulate into an overwrite.
                            nc.tensor.matmul(
                                pso[:, h, :],
                                w_t[:, t, h * 128:(h + 1) * 128],
                                rbf[:, il * B:(il + 1) * B],
                                start=(q == 0 and h == 0),
                                stop=False,
                                skip_group_check=True,
                            )
                        q += 1
                for s in range(3):
                    for h in range(2):
                        nc.tensor.matmul(
                            pso[:, h, :],
                            ws_t[:, s, h * 128:(h + 1) * 128],
                            ls_t[:, s, :],
                            start=False,
                            stop=(s == 2),
                            skip_group_check=True,
                        )

            if loop_n > 0:
                with tc.For_i(0, loop_n, 1):
                    body()
            else:
                for _rep in range(reps):
                    body()
            out_t = const.tile([128, 2, B], f32)
            nc.vector.tensor_copy(out_t[:], pso[:])
            nc.sync.dma_start(out[:], out_t[:])

    nc.compile()
    _nc_cache = nc
    return nc


def make_core_inputs(x, grid, weights, silu_weight, silu_bias):
    """Host-side shard + layout prep. Returns list of 8 input dicts."""
    x = np.ascontiguousarray(x, dtype=np.float32)
    grid = np.ascontiguousarray(grid, dtype=np.float32)
    weights = np.ascontiguousarray(weights, dtype=np.float32)
    silu_weight = np.ascontiguousarray(silu_weight, dtype=np.float32)
    silu_bias = np.ascontiguousarray(silu_bias, dtype=np.float32)

    import ml_dtypes

    def split24(a6, pattern):
        hi = a6.astype(ml_dtypes.bfloat16)
        lo = (a6 - hi.astype(np.float32)).astype(ml_dtypes.bfloat16)
        parts = {"h": hi, "l": lo}
        return np.ascontiguousarray(
            np.concatenate([parts[p] for p in pattern], axis=0)
        )

    # xa: (6, I*B), column j = i*B + b
    xt = x.transpose(1, 0, 2)                       # (I, B, X)
    xa = np.empty((6, IB), dtype=np.float32)
    xa[0:4] = xt.reshape(IB, X).T
    xa[4] = 1.0
    xa[5] = -(xt ** 2).sum(-1).reshape(IB)
    xa24 = split24(xa, "hhll")

    # silu lhsT (core 0 only): rows k2 = i*4+y -> silu(x)[b,i,y]; row 256 -> 1
    sx = x / (1.0 + np.exp(-x))                     # silu(x), (B, I, X)
    lsf = np.zeros((384, B), dtype=np.float32)
    lsf[0:256] = sx.transpose(1, 2, 0).reshape(256, B)
    lsf[256] = 1.0
    ls0 = np.ascontiguousarray(
        lsf.reshape(3, 128, B).transpose(1, 0, 2)).astype(np.float16)
    lsz = np.zeros_like(ls0)

    # silu rhs: M2[(i,y),(o,z)] = sum_x silu_weight[i,o,x]*C[x,y,z]; row 256 bias
    C = _cayley()
    m2 = np.einsum("iox,xyz->iyoz", silu_weight, C).reshape(256, OX)
    wsf = np.zeros((384, OX), dtype=np.float32)
    wsf[0:256] = m2
    wsf[256] = silu_bias.sum(axis=0).reshape(OX)
    ws = np.ascontiguousarray(
        wsf.reshape(3, 128, OX).transpose(1, 0, 2)).astype(np.float16)

    in_maps = []
    for c in range(NCORES):
        gsl = slice(c * GS, (c + 1) * GS)
        gc = grid[gsl]                              # (GS, 4)
        ga = np.empty((6, GS), dtype=np.float32)
        ga[0:4] = 2.0 * gc.T
        ga[4] = -(gc ** 2).sum(-1)
        ga[5] = 1.0
        ga24 = split24(ga, "hlhl")

        # W slab -> [blk, p, t, c] with k-tile q = gb*I + i, rows p = g in block
        warr = weights[:, :, gsl, :].transpose(0, 2, 1, 3).reshape(I, GS, OX)
        tmp = warr.reshape(I, NGB, 128, OX).transpose(1, 0, 2, 3)
        tmp = tmp.reshape(NBLK, TPB, 128, OX).transpose(0, 2, 1, 3)
        wtc = np.ascontiguousarray(tmp).astype(ml_dtypes.float8_e3m4)

        in_maps.append({
            "wt": wtc,
            "ga": ga24,
            "xa": xa24,
            "ls": ls0 if c == 0 else lsz,
            "ws": ws,
        })
    return in_maps


def kernel(x, grid, weights, silu_weight, silu_bias):
    global last_results
    nc = _build_bass()
    in_maps = make_core_inputs(x, grid, weights, silu_weight, silu_bias)
    res = run_bass_kernel_spmd(nc, in_maps, list(range(NCORES)))
    last_results = res
    acc = np.zeros((B, OX), dtype=np.float32)
    for r in res.results:
        o = r["out"]                                # (128, 2, B)
        acc += o.transpose(2, 1, 0).reshape(B, OX)
    return acc.reshape(B, O, X)


# revision 5
# speedup vs baseline: 1.1544x; 1.1187x over previous
"""Trainium2 Bass kernel for the CliffordKAN layer problem (fp8 rework).

Restructure vs kernel.py: the big contraction's stationary operand is a
(128 g, 128 ox-half) slab of W in fp8-e3m4 (1 byte -> halves HBM
traffic), and the moving operand is the fp16 rbf chunk (128 g, 64 b).
PSUM output is (128 ox-half, 64 b) -> the PE runs with all 128 output
partitions occupied (baseline used only 64 = B), i.e. 2x MACs/cycle.
Per k-tile: 2 matmuls (ox halves) x 64 moving rows = 128 cycles vs the
baseline's 256.

Host reassembles out[oxl, h, b] -> out[b, 128h+oxl].

Vs the fp16 baseline: W is stored/streamed as fp8-e3m4 (4 mantissa
bits; rel err ~1.1e-2, gate 2e-2), halving HBM traffic 16.8->8.4 MB
per core, and the big matmuls run with 128 PSUM output partitions
(stationary = W half-tile, moving = rbf chunk) so the moving-stream
total halves: 32768 rows vs 65536.  Measured steady state (For_i
delta, 8 tunneled cores): 41.7 us/iter vs 46.4 baseline; single-shot
stream-bound estimate ~18-20 us.
"""

import numpy as np

from concourse import bacc, bass, mybir  # noqa: F401
from concourse.bass_utils import run_bass_kernel_spmd
from concourse.tile import TileContext

B, I, O, G, X = 64, 64, 64, 4096, 4
NCORES = 8
GS = G // NCORES            # grid points per core = 512
NGB = GS // 128             # g-blocks per core = 4
NKT = NGB * I               # big-matmul k-tiles per core = 256
TPB = 32                    # k-tiles per DMA batch (= 1 MB in fp8)
NBLK = NKT // TPB           # 8 weight DMA batches
OX = O * X                  # 256
IB = I * B                  # 4096
NCH = IB // 512             # rbf chunks (N=512 matmuls) per g-block = 8
IPC = 512 // B              # i's per rbf chunk = 8

_nc_cache = None
last_results = None


def _cayley():
    C = np.zeros((4, 4, 4), dtype=np.float32)
    entries = [
        (0, 0, 0, 1), (0, 1, 1, 1), (0, 2, 2, 1), (0, 3, 3, 1),
        (1, 0, 1, 1), (1, 1, 0, 1), (1, 2, 3, 1), (1, 3, 2, 1),
        (2, 0, 2, 1), (2, 1, 3, -1), (2, 2, 0, 1), (2, 3, 1, -1),
        (3, 0, 3, 1), (3, 1, 2, -1), (3, 2, 1, 1), (3, 3, 0, -1),
    ]
    for xx, y, z, s in entries:
        C[xx, y, z] = s
    return C


def _build_bass(reps=1, loop_n=0):
    global _nc_cache
    if reps == 1 and loop_n == 0 and _nc_cache is not None:
        return _nc_cache

    nc = bacc.Bacc(
        "TRN2", target_bir_lowering=False, debug=False, num_devices=NCORES
    )
    f32 = mybir.dt.float32
    bf16 = mybir.dt.bfloat16
    f16 = mybir.dt.float16
    f8 = mybir.dt.float8e3

    wt = nc.dram_tensor("wt", [NBLK, 128, TPB, OX], f8, kind="ExternalInput")
    ga = nc.dram_tensor("ga", [24, GS], bf16, kind="ExternalInput")
    xa = nc.dram_tensor("xa", [24, IB], bf16, kind="ExternalInput")
    ls = nc.dram_tensor("ls", [128, 3, B], f16, kind="ExternalInput")
    ws = nc.dram_tensor("ws", [128, 3, OX], f16, kind="ExternalInput")
    out = nc.dram_tensor("out", [128, 2, B], f32, kind="ExternalOutput")

    with TileContext(nc) as tc:
        with (
            tc.tile_pool(name="const", bufs=1) as const,
            tc.tile_pool(name="wpool", bufs=8) as wpool,
            tc.tile_pool(name="rpool", bufs=4) as rpool,
            tc.tile_pool(name="psa", bufs=4, space="PSUM") as psa_pool,
            tc.tile_pool(name="pso", bufs=1, space="PSUM") as pso_pool,
        ):
            ga_t = const.tile([24, GS], bf16)
            nc.sync.dma_start(ga_t[:], ga[:])
            xa_t = const.tile([24, IB], bf16)
            nc.sync.dma_start(xa_t[:], xa[:])
            ls_t = const.tile([128, 3, B], f16)
            nc.sync.dma_start(ls_t[:], ls[:])
            ws_t = const.tile([128, 3, OX], f16)
            nc.sync.dma_start(ws_t[:], ws[:])

            pso = pso_pool.tile([128, 2, B], f32)

            def small_chunk(gb, nb):
                psa = psa_pool.tile([128, 512], f32)
                nc.tensor.matmul(
                    psa[:],
                    ga_t[:, gb * 128:(gb + 1) * 128],
                    xa_t[:, nb * 512:(nb + 1) * 512],
                    start=True,
                    stop=True,
                )
                rbf = rpool.tile([128, 512], f16)
                nc.scalar.activation(
                    rbf[:], psa[:], mybir.ActivationFunctionType.Exp
                )
                return rbf

            chunks = [(gb, nb) for gb in range(NGB) for nb in range(NCH)]

            def body():
                # 3-chunk rbf lookahead: ScalarE's exp (~600ns) is as long
                # as a chunk's PE work, so shallow pipelining leaves exp on
                # the critical path between chunks.
                rbf_q = [small_chunk(*chunks[j]) for j in range(3)]
                q = 0
                w_t = None
                for idx in range(len(chunks)):
                    rbf = rbf_q.pop(0)
                    if idx + 3 < len(chunks):
                        rbf_q.append(small_chunk(*chunks[idx + 3]))
                    for il in range(IPC):
                        blk, t = divmod(q, TPB)
                        if t == 0:
                            w_t = wpool.tile([128, TPB, OX], f8)
                            nc.sync.dma_start(w_t[:], wt[blk])
                        for h in range(2):
                            # one start only: the pending-zero marking is
                            # per 2KB zero region, so a second start=True
                            # (h=1) would re-mark h=0's bytes and turn the
                            # next h=0 accumulate into an overwrite.
                            nc.tensor.matmul(
                                pso[:, h, :],
                                w_t[:, t, h * 128:(h + 1) * 128],
                                rbf[:, il * B:(il + 1) * B],
                                start=(q == 0 and h == 0),
                                stop=False,
                                skip_group_check=True,
                            )
                        q += 1
                for s in range(3):
                    for h in range(2):
                        nc.tensor.matmul(
                            pso[:, h, :],
                            ws_t[:, s, h * 128:(h + 1) * 128],
                            ls_t[:, s, :],
                            start=False,
                            stop=(s == 2),
                            skip_group_check=True,
                        )

            if loop_n > 0:
                with tc.For_i(0, loop_n, 1):
                    body()
            else:
                for _rep in range(reps):
                    body()
            out_t = const.tile([128, 2, B], f32)
            nc.vector.tensor_copy(out_t[:], pso[:])
            nc.sync.dma_start(out[:], out_t[:])

    nc.compile()
    _nc_cache = nc
    return nc


def make_core_inputs(x, grid, weights, silu_weight, silu_bias):
    """Host-side shard + layout prep. Returns list of 8 input dicts."""
    x = np.ascontiguousarray(x, dtype=np.float32)
    grid = np.ascontiguousarray(grid, dtype=np.float32)
    weights = np.ascontiguousarray(weights, dtype=np.float32)
    silu_weight = np.ascontiguousarray(silu_weight, dtype=np.float32)
    silu_bias = np.ascontiguousarray(silu_bias, dtype=np.float32)

    import ml_dtypes

    def split24(a6, pattern):
        hi = a6.astype(ml_dtypes.bfloat16)
        lo = (a6 - hi.astype(np.float32)).astype(ml_dtypes.bfloat16)
        parts = {"h": hi, "l": lo}
        return np.ascontiguousarray(
            np.concatenate([parts[p] for p in pattern], axis=0)
        )

    # xa: (6, I*B), column j = i*B + b
    xt = x.transpose(1, 0, 2)                       # (I, B, X)
    xa = np.empty((6, IB), dtype=np.float32)
    xa[0:4] = xt.reshape(IB, X).T
    xa[4] = 1.0
    xa[5] = -(xt ** 2).sum(-1).reshape(IB)
    xa24 = split24(xa, "hhll")

    # silu lhsT (core 0 only): rows k2 = i*4+y -> silu(x)[b,i,y]; row 256 -> 1
    sx = x / (1.0 + np.exp(-x))                     # silu(x), (B, I, X)
    lsf = np.zeros((384, B), dtype=np.float32)
    lsf[0:256] = sx.transpose(1, 2, 0).reshape(256, B)
    lsf[256] = 1.0
    ls0 = np.ascontiguousarray(
        lsf.reshape(3, 128, B).transpose(1, 0, 2)).astype(np.float16)
    lsz = np.zeros_like(ls0)

    # silu rhs: M2[(i,y),(o,z)] = sum_x silu_weight[i,o,x]*C[x,y,z]; row 256 bias
    C = _cayley()
    m2 = np.einsum("iox,xyz->iyoz", silu_weight, C).reshape(256, OX)
    wsf = np.zeros((384, OX), dtype=np.float32)
    wsf[0:256] = m2
    wsf[256] = silu_bias.sum(axis=0).reshape(OX)
    ws = np.ascontiguousarray(
        wsf.reshape(3, 128, OX).transpose(1, 0, 2)).astype(np.float16)

    in_maps = []
    for c in range(NCORES):
        gsl = slice(c * GS, (c + 1) * GS)
        gc = grid[gsl]                              # (GS, 4)
        ga = np.empty((6, GS), dtype=np.float32)
        ga[0:4] = 2.0 * gc.T
        ga[4] = -(gc ** 2).sum(-1)
        ga[5] = 1.0
        ga24 = split24(ga, "hlhl")

        # W slab -> [blk, p, t, c] with k-tile q = gb*I + i, rows p = g in block
        warr = weights[:, :, gsl, :].transpose(0, 2, 1, 3).reshape(I, GS, OX)
        tmp = warr.reshape(I, NGB, 128, OX).transpose(1, 0, 2, 3)
        tmp = tmp.reshape(NBLK, TPB, 128, OX).transpose(0, 2, 1, 3)
        wtc = np.ascontiguousarray(tmp).astype(ml_dtypes.float8_e3m4)

        in_maps.append({
            "wt": wtc,
            "ga": ga24,
            "xa": xa24,
            "ls": ls0 if c == 0 else lsz,
            "ws": ws,
        })
    return in_maps


def kernel(x, grid, weights, silu_weight, silu_bias):
    global last_results
    nc = _build_bass()
    in_maps = make_core_inputs(x, grid, weights, silu_weight, silu_bias)
    res = run_bass_kernel_spmd(nc, in_maps, list(range(NCORES)))
    last_results = res
    acc = np.zeros((B, OX), dtype=np.float32)
    for r in res.results:
        o = r["out"]                                # (128, 2, B)
        acc += o.transpose(2, 1, 0).reshape(B, OX)
    return acc.reshape(B, O, X)


# revision 6
# speedup vs baseline: 1.2545x; 1.0867x over previous
"""Trainium2 Bass kernel for the CliffordKAN layer problem (fp8 rework).

Restructure vs kernel.py: the big contraction's stationary operand is a
(128 g, 128 ox-half) slab of W in fp8-e3m4 (1 byte -> halves HBM
traffic), and the moving operand is the fp16 rbf chunk (128 g, 64 b).
PSUM output is (128 ox-half, 64 b) -> the PE runs with all 128 output
partitions occupied (baseline used only 64 = B), i.e. 2x MACs/cycle.
Per k-tile: 2 matmuls (ox halves) x 64 moving rows = 128 cycles vs the
baseline's 256.

Host reassembles out[oxl, h, b] -> out[b, 128h+oxl].

Vs the fp16 baseline: W is stored/streamed as fp8-e3m4 (4 mantissa
bits; rel err ~1.1e-2, gate 2e-2), halving HBM traffic 16.8->8.4 MB
per core, and the big matmuls run with 128 PSUM output partitions
(stationary = W half-tile, moving = rbf chunk) so the moving-stream
total halves: 32768 rows vs 65536.  Measured steady state (For_i
delta, 8 tunneled cores): 41.7 us/iter vs 46.4 baseline; single-shot
stream-bound estimate ~18-20 us.
"""

import numpy as np

from concourse import bacc, bass, mybir  # noqa: F401
from concourse.bass_utils import run_bass_kernel_spmd
from concourse.tile import TileContext

B, I, O, G, X = 64, 64, 64, 4096, 4
NCORES = 8
GS = G // NCORES            # grid points per core = 512
NGB = GS // 128             # g-blocks per core = 4
NKT = NGB * I               # big-matmul k-tiles per core = 256
TPB = 32                    # k-tiles per DMA batch (= 1 MB in fp8)
NBLK = NKT // TPB           # 8 weight DMA batches
OX = O * X                  # 256
IB = I * B                  # 4096
NCH = IB // 512             # rbf chunks (N=512 matmuls) per g-block = 8
IPC = 512 // B              # i's per rbf chunk = 8

_nc_cache = None
last_results = None


def _cayley():
    C = np.zeros((4, 4, 4), dtype=np.float32)
    entries = [
        (0, 0, 0, 1), (0, 1, 1, 1), (0, 2, 2, 1), (0, 3, 3, 1),
        (1, 0, 1, 1), (1, 1, 0, 1), (1, 2, 3, 1), (1, 3, 2, 1),
        (2, 0, 2, 1), (2, 1, 3, -1), (2, 2, 0, 1), (2, 3, 1, -1),
        (3, 0, 3, 1), (3, 1, 2, -1), (3, 2, 1, 1), (3, 3, 0, -1),
    ]
    for xx, y, z, s in entries:
        C[xx, y, z] = s
    return C


def _build_bass(reps=1, loop_n=0):
    global _nc_cache
    if reps == 1 and loop_n == 0 and _nc_cache is not None:
        return _nc_cache

    nc = bacc.Bacc(
        "TRN2", target_bir_lowering=False, debug=False, num_devices=NCORES
    )
    f32 = mybir.dt.float32
    bf16 = mybir.dt.bfloat16
    f16 = mybir.dt.float16
    f8 = mybir.dt.float8e3

    wt = nc.dram_tensor("wt", [NBLK, 128, TPB, OX], f8, kind="ExternalInput")
    ga = nc.dram_tensor("ga", [24, GS], bf16, kind="ExternalInput")
    xa = nc.dram_tensor("xa", [24, IB], bf16, kind="ExternalInput")
    ls = nc.dram_tensor("ls", [128, 3, B], f16, kind="ExternalInput")
    ws = nc.dram_tensor("ws", [128, 3, OX], f16, kind="ExternalInput")
    out = nc.dram_tensor("out", [128, 2, B], f32, kind="ExternalOutput")

    with TileContext(nc) as tc:
        with (
            tc.tile_pool(name="const", bufs=1) as const,
            tc.tile_pool(name="wpool", bufs=8) as wpool,
            tc.tile_pool(name="rpool", bufs=5) as rpool,
            tc.tile_pool(name="psa", bufs=5, space="PSUM") as psa_pool,
            tc.tile_pool(name="pso", bufs=1, space="PSUM") as pso_pool,
        ):
            ga_t = const.tile([24, GS], bf16)
            nc.sync.dma_start(ga_t[:], ga[:])
            xa_t = const.tile([24, IB], bf16)
            nc.sync.dma_start(xa_t[:], xa[:])
            ls_t = const.tile([128, 3, B], f16)
            nc.sync.dma_start(ls_t[:], ls[:])
            ws_t = const.tile([128, 3, OX], f16)
            nc.sync.dma_start(ws_t[:], ws[:])

            pso = pso_pool.tile([128, 2, B], f32)

            def small_chunk(gb, nb):
                psa = psa_pool.tile([128, 512], f32)
                nc.tensor.matmul(
                    psa[:],
                    ga_t[:, gb * 128:(gb + 1) * 128],
                    xa_t[:, nb * 512:(nb + 1) * 512],
                    start=True,
                    stop=True,
                )
                rbf = rpool.tile([128, 512], f16)
                nc.scalar.activation(
                    rbf[:], psa[:], mybir.ActivationFunctionType.Exp
                )
                return rbf

            chunks = [(gb, nb) for gb in range(NGB) for nb in range(NCH)]

            def body():
                # 3-chunk rbf lookahead: ScalarE's exp (~600ns) is as long
                # as a chunk's PE work, so shallow pipelining leaves exp on
                # the critical path between chunks.
                rbf_q = [small_chunk(*chunks[j]) for j in range(4)]
                q = 0
                w_t = None
                for idx in range(len(chunks)):
                    rbf = rbf_q.pop(0)
                    if idx + 4 < len(chunks):
                        rbf_q.append(small_chunk(*chunks[idx + 4]))
                    for il in range(IPC):
                        blk, t = divmod(q, TPB)
                        if t == 0:
                            w_t = wpool.tile([128, TPB, OX], f8)
                            nc.sync.dma_start(w_t[:], wt[blk])
                        for h in range(2):
                            # one start only: the pending-zero marking is
                            # per 2KB zero region, so a second start=True
                            # (h=1) would re-mark h=0's bytes and turn the
                            # next h=0 accumulate into an overwrite.
                            nc.tensor.matmul(
                                pso[:, h, :],
                                w_t[:, t, h * 128:(h + 1) * 128],
                                rbf[:, il * B:(il + 1) * B],
                                start=(q == 0 and h == 0),
                                stop=False,
                                skip_group_check=True,
                            )
                        q += 1
                for s in range(3):
                    for h in range(2):
                        nc.tensor.matmul(
                            pso[:, h, :],
                            ws_t[:, s, h * 128:(h + 1) * 128],
                            ls_t[:, s, :],
                            start=False,
                            stop=(s == 2),
                            skip_group_check=True,
                        )

            if loop_n > 0:
                with tc.For_i(0, loop_n, 1):
                    body()
            else:
                for _rep in range(reps):
                    body()
            out_t = const.tile([128, 2, B], f32)
            nc.vector.tensor_copy(out_t[:], pso[:])
            nc.sync.dma_start(out[:], out_t[:])

    nc.compile()
    _nc_cache = nc
    return nc


def make_core_inputs(x, grid, weights, silu_weight, silu_bias):
    """Host-side shard + layout prep. Returns list of 8 input dicts."""
    x = np.ascontiguousarray(x, dtype=np.float32)
    grid = np.ascontiguousarray(grid, dtype=np.float32)
    weights = np.ascontiguousarray(weights, dtype=np.float32)
    silu_weight = np.ascontiguousarray(silu_weight, dtype=np.float32)
    silu_bias = np.ascontiguousarray(silu_bias, dtype=np.float32)

    import ml_dtypes

    def split24(a6, pattern):
        hi = a6.astype(ml_dtypes.bfloat16)
        lo = (a6 - hi.astype(np.float32)).astype(ml_dtypes.bfloat16)
        parts = {"h": hi, "l": lo}
        return np.ascontiguousarray(
            np.concatenate([parts[p] for p in pattern], axis=0)
        )

    # xa: (6, I*B), column j = i*B + b
    xt = x.transpose(1, 0, 2)                       # (I, B, X)
    xa = np.empty((6, IB), dtype=np.float32)
    xa[0:4] = xt.reshape(IB, X).T
    xa[4] = 1.0
    xa[5] = -(xt ** 2).sum(-1).reshape(IB)
    xa24 = split24(xa, "hhll")

    # silu lhsT (core 0 only): rows k2 = i*4+y -> silu(x)[b,i,y]; row 256 -> 1
    sx = x / (1.0 + np.exp(-x))                     # silu(x), (B, I, X)
    lsf = np.zeros((384, B), dtype=np.float32)
    lsf[0:256] = sx.transpose(1, 2, 0).reshape(256, B)
    lsf[256] = 1.0
    ls0 = np.ascontiguousarray(
        lsf.reshape(3, 128, B).transpose(1, 0, 2)).astype(np.float16)
    lsz = np.zeros_like(ls0)

    # silu rhs: M2[(i,y),(o,z)] = sum_x silu_weight[i,o,x]*C[x,y,z]; row 256 bias
    C = _cayley()
    m2 = np.einsum("iox,xyz->iyoz", silu_weight, C).reshape(256, OX)
    wsf = np.zeros((384, OX), dtype=np.float32)
    wsf[0:256] = m2
    wsf[256] = silu_bias.sum(axis=0).reshape(OX)
    ws = np.ascontiguousarray(
        wsf.reshape(3, 128, OX).transpose(1, 0, 2)).astype(np.float16)

    in_maps = []
    for c in range(NCORES):
        gsl = slice(c * GS, (c + 1) * GS)
        gc = grid[gsl]                              # (GS, 4)
        ga = np.empty((6, GS), dtype=np.float32)
        ga[0:4] = 2.0 * gc.T
        ga[4] = -(gc ** 2).sum(-1)
        ga[5] = 1.0
        ga24 = split24(ga, "hlhl")

        # W slab -> [blk, p, t, c] with k-tile q = gb*I + i, rows p = g in block
        warr = weights[:, :, gsl, :].transpose(0, 2, 1, 3).reshape(I, GS, OX)
        tmp = warr.reshape(I, NGB, 128, OX).transpose(1, 0, 2, 3)
        tmp = tmp.reshape(NBLK, TPB, 128, OX).transpose(0, 2, 1, 3)
        wtc = np.ascontiguousarray(tmp).astype(ml_dtypes.float8_e3m4)

        in_maps.append({
            "wt": wtc,
            "ga": ga24,
            "xa": xa24,
            "ls": ls0 if c == 0 else lsz,
            "ws": ws,
        })
    return in_maps


def kernel(x, grid, weights, silu_weight, silu_bias):
    global last_results
    nc = _build_bass()
    in_maps = make_core_inputs(x, grid, weights, silu_weight, silu_bias)
    res = run_bass_kernel_spmd(nc, in_maps, list(range(NCORES)))
    last_results = res
    acc = np.zeros((B, OX), dtype=np.float32)
    for r in res.results:
        o = r["out"]                                # (128, 2, B)
        acc += o.transpose(2, 1, 0).reshape(B, OX)
    return acc.reshape(B, O, X)
